# revision 1
# baseline (speedup 1.0000x reference)
"""BAG-LSTM fused kernel for Trainium2 (Bass/Tile), data-parallel over 8 cores.

Layout strategy (per core, batch shard BL=1024 rows):
- Batch on SBUF partitions everywhere; features on the free dim, so LSTM
  masks / norms / LayerNorm are per-partition scalars and free-dim reduces.
- All GEMMs in float32r (tf32-class: measured ~1.7e-4 rel absmax vs fp64 at
  K=512, same PE speed as bf16, 4x faster than fp32). f32r operands must be
  *produced* as f32r (DMA-cast via gpsimd or copy-cast at PSUM evac).
- Stationary operands are transposed activations (X.T tiles) built with
  PE transpose-mode vs an identity; moving operands are weight tiles
  streamed from DRAM with gpsimd cast-DMA.
- Biases are added inside the GEMM accumulation as a K=1 matmul
  (ones[1,128] x bias_row[1,N]).
- LSTM streams a_W/v_W once in 256-wide gate stripes (i|f|g|o columns of the
  same 256-feature block together) so the cell math consumes gates
  immediately; c / o(sigmoid) / c^T spill to DRAM scratch between phases.
- BAG keeps W_mb/W_b resident (f32r), reloads c / c^T / o per m-tile,
  and fuses everything with scalar_tensor_tensor (+free accum_out for the
  norms/mean), dual-scalar tensor_scalar for (x-mu)*rstd, and ACT
  sigmoid/tanh/relu/sqrt/rsqrt evacuations.

Known-good toolchain facts this file relies on (measured in this container):
- bacc.Bacc + nc.compile() legalizes the 1-sync-wait-per-instruction HW
  constraint (raw bass.Bass fails walrus codegen).
- memset cannot write float32r; copy-cast from f32 instead.
- gpsimd cannot touch PSUM.

The module builds one SPMD NEFF and runs it on cores 0..7 with
batch-sharded inputs; weights are replicated.
"""
import sys

import numpy as np

try:
    import concourse.bacc as bacc
except ImportError:  # fresh-dir grading: repo comes from the container env
    sys.path.insert(0, "/opt/trn_rl_repo")
    import concourse.bacc as bacc

import concourse.mybir as mybir
import concourse.tile as tile
from concourse.bass_utils import run_bass_kernel_spmd
from concourse.masks import make_identity
from contextlib import ExitStack

F32 = mybir.dt.float32
F32R = mybir.dt.float32r
Act = mybir.ActivationFunctionType
Alu = mybir.AluOpType

NCORES = 8
B, H = 8192, 1024
BL = B // NCORES          # 1024 batch rows per core
MT = BL // 128            # 8 m-tiles
KT1 = H // 128            # 8  k-tiles for H contraction
KT2 = 2 * H // 128        # 16 k-tiles for 2H contraction
JB = 4                    # LSTM feature blocks per gate
JW = H // JB              # 256 block width
LN_EPS = 1e-5
BAG_EPS = 1e-6


def build():
    nc = bacc.Bacc("TRN2", target_bir_lowering=False, debug=False)

    def din(name, shape):
        return nc.dram_tensor(name, shape, F32, kind="ExternalInput")

    def dout(name, shape):
        return nc.dram_tensor(name, shape, F32, kind="ExternalOutput")

    a_x, a_h0, a_c0 = din("a_x", [BL, H]), din("a_h0", [BL, H]), din("a_c0", [BL, H])
    v_x, v_h0, v_c0 = din("v_x", [BL, H]), din("v_h0", [BL, H]), din("v_c0", [BL, H])
    aco = din("aco_is_rnn_list", [BL, 1])
    vis = din("vis_is_rnn_list", [BL, 1])
    isb = din("is_bag_list", [BL, 1])
    a_W, a_b = din("a_W", [2 * H, 4 * H]), din("a_b", [4 * H])
    v_W, v_b = din("v_W", [2 * H, 4 * H]), din("v_b", [4 * H])
    W_mb, b_mb = din("W_mb", [2 * H, H]), din("b_mb", [H])
    W_b, b_b = din("W_b", [H, H]), din("b_b", [H])
    ln_g, ln_b = din("ln_g", [H]), din("ln_b", [H])

    a_h, a_sc = dout("a_h", [BL, H]), dout("a_sc", [BL, H])
    v_h, v_sc = dout("v_h", [BL, H]), dout("v_sc", [BL, H])

    # DRAM scratch (per core)
    c_scr = {k: nc.dram_tensor(f"c_{k}_scr", [BL, H], F32) for k in ("a", "v")}
    o_scr = {k: nc.dram_tensor(f"o_{k}_scr", [BL, H], F32) for k in ("a", "v")}
    ct_scr = {k: nc.dram_tensor(f"ct_{k}_scr", [128, KT1, MT, 128], F32R)
              for k in ("a", "v")}

    with tile.TileContext(nc) as tc, ExitStack() as ctx:
        consts = ctx.enter_context(tc.tile_pool(name="consts", bufs=1))
        stats = ctx.enter_context(tc.tile_pool(name="stats", bufs=24))

        ident = consts.tile([128, 128], F32)
        make_identity(nc, ident)
        ones_f = consts.tile([1, 128], F32)
        nc.vector.memset(ones_f[:], 1.0)
        ones = consts.tile([1, 128], F32R)
        nc.vector.tensor_copy(out=ones[:], in_=ones_f[:])

        # per-partition masks [128, MT]: column m = batch rows m*128..m*128+127
        def load_mask(dram):
            t = consts.tile([128, MT], F32, tag=f"mask_{dram.name}")
            nc.sync.dma_start(out=t[:], in_=dram[:].rearrange("(m p) o -> p (m o)", p=128))
            return t

        aco_m = load_mask(aco)
        vis_m = load_mask(vis)
        isb_m = load_mask(isb)
        # 1 - mask
        aco_om = consts.tile([128, MT], F32, tag="aco_om")
        vis_om = consts.tile([128, MT], F32, tag="vis_om")
        nc.vector.tensor_scalar(out=aco_om[:], in0=aco_m[:], scalar1=-1.0,
                                scalar2=1.0, op0=Alu.mult, op1=Alu.add)
        nc.vector.tensor_scalar(out=vis_om[:], in0=vis_m[:], scalar1=-1.0,
                                scalar2=1.0, op0=Alu.mult, op1=Alu.add)

        # ---------------- LSTM phase (run twice: a then v) ----------------
        # W is streamed once in [2048, 512] gate-half slabs, order
        # i,g,f,o per 512-col half, so the cell math consumes each gate
        # immediately: P accumulates i then i*g; f-slab finishes c; o spills.
        def lstm_phase(tag, x_in, h0_in, c0_in, W_in, b_in, m_col, om_col):
            with ExitStack() as ph:
                xtp = ph.enter_context(tc.tile_pool(name=f"xt_{tag}", bufs=1))
                wlp = ph.enter_context(tc.tile_pool(name=f"wl_{tag}", bufs=4))
                xrp = ph.enter_context(tc.tile_pool(name=f"xr_{tag}", bufs=2))
                pap = ph.enter_context(tc.tile_pool(name=f"pa_{tag}", bufs=2))
                c0p = ph.enter_context(tc.tile_pool(name=f"c0_{tag}", bufs=2))
                gep = ph.enter_context(tc.tile_pool(name=f"ge_{tag}", bufs=3))
                ccp = ph.enter_context(tc.tile_pool(name=f"cc_{tag}", bufs=2))
                ctev = ph.enter_context(tc.tile_pool(name=f"ctv_{tag}", bufs=4))
                bp = ph.enter_context(tc.tile_pool(name=f"bp_{tag}", bufs=2))
                gps = ph.enter_context(tc.tile_pool(name=f"gp_{tag}", bufs=6,
                                                    space="PSUM"))
                tps = ph.enter_context(tc.tile_pool(name=f"tp_{tag}", bufs=2,
                                                    space="PSUM"))

                with nc.named_scope(f"xt_{tag}"):
                    # X.T tiles: k 0..7 from x, 8..15 from h0  -> f32r
                    xt = xtp.tile([128, KT2, MT, 128], F32R, tag="xt")
                    for src, kofs in ((x_in, 0), (h0_in, KT1)):
                        for m in range(MT):
                            xr = xrp.tile([128, H], F32, tag="xrow")
                            nc.sync.dma_start(out=xr[:],
                                              in_=src[m * 128:(m + 1) * 128, :])
                            for k in range(KT1):
                                tp = tps.tile([128, 128], F32, tag="tp")
                                nc.tensor.transpose(
                                    tp[:], xr[:, k * 128:(k + 1) * 128], ident[:])
                                nc.scalar.copy(out=xt[:, kofs + k, m, :], in_=tp[:])

                with nc.named_scope(f"lstm_{tag}"):
                    for ns in range(2):
                        pacc = pap.tile([128, MT, 512], F32, tag="pacc")
                        for gate in (0, 2, 1, 3):      # i, g, f, o
                            cols = gate * H + ns * 512
                            wt_lo = wlp.tile([128, KT1, 512], F32R, tag="wslab")
                            nc.scalar.dma_start(
                                out=wt_lo[:],
                                in_=W_in[:H, cols:cols + 512].rearrange(
                                    "(k p) c -> p k c", p=128).bitcast(F32R))
                            wt_hi = wlp.tile([128, KT1, 512], F32R, tag="wslab")
                            nc.scalar.dma_start(
                                out=wt_hi[:],
                                in_=W_in[H:, cols:cols + 512].rearrange(
                                    "(k p) c -> p k c", p=128).bitcast(F32R))
                            bt = bp.tile([128, 512], F32, tag="brow")
                            nc.sync.dma_start(
                                out=bt[:],
                                in_=b_in[cols:cols + 512].unsqueeze(0)
                                .partition_broadcast(128).squeeze(1))
                            for m in range(MT):
                                pt = gps.tile([128, 512], F32, tag="gpt")
                                for k in range(KT2):
                                    wsrc = wt_lo if k < KT1 else wt_hi
                                    nc.tensor.matmul(pt[:], xt[:, k, m, :],
                                                     wsrc[:, k % KT1, :],
                                                     start=(k == 0),
                                                     stop=(k == KT2 - 1))
                                # bias add on DVE (PSUM + broadcast row), then
                                # the activation evac reads SBUF
                                gb = gep.tile([128, 512], F32, tag="gb")
                                nc.vector.tensor_add(gb[:], pt[:], bt[:])
                                if gate == 0:          # i -> P
                                    nc.scalar.activation(out=pacc[:, m, :],
                                                         in_=gb[:],
                                                         func=Act.Sigmoid)
                                elif gate == 2:        # g: P *= tanh(g)
                                    nc.scalar.activation(out=gb[:], in_=gb[:],
                                                         func=Act.Tanh)
                                    nc.vector.tensor_mul(pacc[:, m, :],
                                                         pacc[:, m, :], gb[:])
                                elif gate == 1:        # f: finish c
                                    nc.scalar.activation(out=gb[:], in_=gb[:],
                                                         func=Act.Sigmoid)
                                    nc.vector.tensor_scalar(
                                        out=gb[:], in0=gb[:],
                                        scalar1=m_col[:, m:m + 1],
                                        scalar2=om_col[:, m:m + 1],
                                        op0=Alu.mult, op1=Alu.add)
                                    c0b = c0p.tile([128, 512], F32, tag="c0b")
                                    nc.sync.dma_start(
                                        out=c0b[:],
                                        in_=c0_in[m * 128:(m + 1) * 128,
                                                  ns * 512:(ns + 1) * 512])
                                    nc.vector.tensor_mul(gb[:], gb[:], c0b[:])
                                    cb = ccp.tile([128, 512], F32, tag="cb")
                                    nc.vector.scalar_tensor_tensor(
                                        out=cb[:], in0=pacc[:, m, :],
                                        scalar=m_col[:, m:m + 1], in1=gb[:],
                                        op0=Alu.mult, op1=Alu.add)
                                    nc.sync.dma_start(
                                        out=c_scr[tag][m * 128:(m + 1) * 128,
                                                       ns * 512:(ns + 1) * 512],
                                        in_=cb[:])
                                    for hh in range(4):
                                        tp = tps.tile([128, 128], F32, tag="tp")
                                        nc.tensor.transpose(
                                            tp[:], cb[:, hh * 128:(hh + 1) * 128],
                                            ident[:])
                                        ct = ctev.tile([128, 128], F32R, tag="ctev")
                                        nc.scalar.copy(out=ct[:], in_=tp[:])
                                        nc.sync.dma_start(
                                            out=ct_scr[tag][:, ns * 4 + hh, m, :],
                                            in_=ct[:])
                                else:                  # o: spill sigmoid(o)
                                    nc.scalar.activation(out=gb[:], in_=gb[:],
                                                         func=Act.Sigmoid)
                                    nc.sync.dma_start(
                                        out=o_scr[tag][m * 128:(m + 1) * 128,
                                                       ns * 512:(ns + 1) * 512],
                                        in_=gb[:])

        lstm_phase("a", a_x, a_h0, a_c0, a_W, a_b, aco_m, aco_om)
        lstm_phase("v", v_x, v_h0, v_c0, v_W, v_b, vis_m, vis_om)

        # ---------------- BAG phase ----------------
        with ExitStack() as ph:
            bwp = ph.enter_context(tc.tile_pool(name="bagw", bufs=1))
            ctp = ph.enter_context(tc.tile_pool(name="bagct", bufs=2))
            cmp_ = ph.enter_context(tc.tile_pool(name="bagcm", bufs=2))
            orp = ph.enter_context(tc.tile_pool(name="bagor", bufs=2))
            wbp = ph.enter_context(tc.tile_pool(name="bagwb", bufs=2))
            hmp = ph.enter_context(tc.tile_pool(name="baghm", bufs=2))
            jkp = ph.enter_context(tc.tile_pool(name="bagjk", bufs=2))
            bps = ph.enter_context(tc.tile_pool(name="bagps", bufs=8, space="PSUM"))

            wmb = bwp.tile([128, KT2, H], F32R, tag="wmb")
            for k in range(KT2):
                nc.scalar.dma_start(out=wmb[:, k, :],
                                    in_=W_mb[k * 128:(k + 1) * 128, :].bitcast(F32R))
            wb_t = bwp.tile([128, KT1, H], F32R, tag="wbt")
            for k in range(KT1):
                nc.scalar.dma_start(out=wb_t[:, k, :],
                                    in_=W_b[k * 128:(k + 1) * 128, :].bitcast(F32R))
            bmb = []
            bbt = []
            for r in range(2):
                t1 = bwp.tile([1, 512], F32R, tag=f"bmb{r}")
                nc.sync.dma_start(out=t1[:],
                                  in_=b_mb[r * 512:(r + 1) * 512].unsqueeze(0).bitcast(F32R))
                bmb.append(t1)
                t2 = bwp.tile([1, 512], F32R, tag=f"bbt{r}")
                nc.sync.dma_start(out=t2[:],
                                  in_=b_b[r * 512:(r + 1) * 512].unsqueeze(0).bitcast(F32R))
                bbt.append(t2)
            lg = bwp.tile([128, H], F32, tag="lg")
            nc.gpsimd.dma_start(out=lg[:], in_=ln_g[:].unsqueeze(0).partition_broadcast(128).squeeze(1))
            lb = bwp.tile([128, H], F32, tag="lb")
            nc.gpsimd.dma_start(out=lb[:], in_=ln_b[:].unsqueeze(0).partition_broadcast(128).squeeze(1))
            epsb = consts.tile([128, 1], F32, tag="epsb")
            nc.vector.memset(epsb[:], BAG_EPS)
            epsl = consts.tile([128, 1], F32, tag="epsl")
            nc.vector.memset(epsl[:], LN_EPS)

            with nc.named_scope("bag"):
                for m in range(MT):
                    cta = ctp.tile([128, KT1, 128], F32R, tag="cta")
                    nc.sync.dma_start(out=cta[:], in_=ct_scr["a"][:, :, m, :])
                    ctv = ctp.tile([128, KT1, 128], F32R, tag="ctv")
                    nc.sync.dma_start(out=ctv[:], in_=ct_scr["v"][:, :, m, :])
                    ca = cmp_.tile([128, H], F32, tag="ca")
                    nc.sync.dma_start(out=ca[:], in_=c_scr["a"][m * 128:(m + 1) * 128, :])
                    cv = cmp_.tile([128, H], F32, tag="cv")
                    nc.sync.dma_start(out=cv[:], in_=c_scr["v"][m * 128:(m + 1) * 128, :])
                    # ||main||^2 hoisted ahead of the GEMMs (dep: cmain only)
                    jk0 = jkp.tile([128, H], F32, tag="jk")
                    ems_a = stats.tile([128, 1], F32, tag="ems")
                    nc.vector.scalar_tensor_tensor(
                        out=jk0[:], in0=ca[:], scalar=1.0, in1=ca[:],
                        op0=Alu.mult, op1=Alu.mult, accum_out=ems_a[:])
                    ems_v = stats.tile([128, 1], F32, tag="ems")
                    nc.vector.scalar_tensor_tensor(
                        out=jk0[:], in0=cv[:], scalar=1.0, in1=cv[:],
                        op0=Alu.mult, op1=Alu.mult, accum_out=ems_v[:])

                    def mb_gemm(first, second, tag):
                        ps = []
                        for ns in range(2):
                            p = bps.tile([128, 512], F32, tag="bps")
                            for k in range(KT2):
                                st = first[:, k, :] if k < KT1 else second[:, k - KT1, :]
                                nc.tensor.matmul(p[:], st, wmb[:, k, ns * 512:(ns + 1) * 512],
                                                 start=(k == 0), stop=False)
                            nc.tensor.matmul(p[:], ones[:], bmb[ns][:],
                                             start=False, stop=True)
                            ps.append(p)
                        return ps

                    def b_gemm(ct, tag):
                        ps = []
                        for ns in range(2):
                            p = bps.tile([128, 512], F32, tag="bps")
                            for k in range(KT1):
                                nc.tensor.matmul(p[:], ct[:, k, :],
                                                 wb_t[:, k, ns * 512:(ns + 1) * 512],
                                                 start=(k == 0), stop=False)
                            nc.tensor.matmul(p[:], ones[:], bbt[ns][:],
                                             start=False, stop=True)
                            ps.append(p)
                        return ps

                    u1 = mb_gemm(cta, ctv, "u1")
                    u2 = mb_gemm(ctv, cta, "u2")
                    w1 = b_gemm(ctv, "w1")
                    w2 = b_gemm(cta, "w2")

                    def bag_half(u, w, main, ems, m_col, om_col, out_sc, side):
                        # weight_b = relu(u); h_m = weight_b * w
                        wbt_ = wbp.tile([128, H], F32, tag="wbrelu")
                        nc.scalar.activation(out=wbt_[:, 0:512], in_=u[0][:], func=Act.Relu)
                        nc.scalar.activation(out=wbt_[:, 512:], in_=u[1][:], func=Act.Relu)
                        hm = hmp.tile([128, H], F32, tag="hm")
                        nc.vector.tensor_mul(hm[:, 0:512], wbt_[:, 0:512], w[0][:])
                        nc.vector.tensor_mul(hm[:, 512:], wbt_[:, 512:], w[1][:])
                        # norms
                        jk = jkp.tile([128, H], F32, tag="jk")
                        hms = stats.tile([128, 1], F32, tag="hms")
                        nc.vector.scalar_tensor_tensor(
                            out=jk[:], in0=hm[:], scalar=1.0, in1=hm[:],
                            op0=Alu.mult, op1=Alu.mult, accum_out=hms[:])
                        emn = stats.tile([128, 1], F32, tag="emn")
                        nc.scalar.activation(out=emn[:], in_=ems[:], func=Act.Sqrt)
                        hmn = stats.tile([128, 1], F32, tag="hmn")
                        nc.scalar.activation(out=hmn[:], in_=hms[:], func=Act.Sqrt)
                        # alpha = min(emn / (hmn + eps), 1)
                        hre = stats.tile([128, 1], F32, tag="hre")
                        nc.vector.tensor_scalar_add(hre[:], hmn[:], epsb[:])
                        nc.vector.reciprocal(out=hre[:], in_=hre[:])
                        alpha = stats.tile([128, 1], F32, tag="alpha")
                        nc.vector.tensor_mul(alpha[:], emn[:], hre[:])
                        nc.vector.tensor_scalar_min(alpha[:], alpha[:], 1.0)
                        # pre = alpha*hm + main  (accum -> sum)
                        s1 = stats.tile([128, 1], F32, tag="s1")
                        nc.vector.scalar_tensor_tensor(
                            out=hm[:], in0=hm[:], scalar=alpha[:], in1=main[:],
                            op0=Alu.mult, op1=Alu.add, accum_out=s1[:])
                        s2 = stats.tile([128, 1], F32, tag="s2")
                        nc.vector.scalar_tensor_tensor(
                            out=jk[:], in0=hm[:], scalar=1.0, in1=hm[:],
                            op0=Alu.mult, op1=Alu.mult, accum_out=s2[:])
                        # mu/var/rstd
                        nmu = stats.tile([128, 1], F32, tag="nmu")
                        nc.vector.tensor_scalar_mul(nmu[:], s1[:], -1.0 / H)
                        var = stats.tile([128, 1], F32, tag="var")
                        nc.vector.tensor_scalar_mul(var[:], s2[:], 1.0 / H)
                        mu2 = stats.tile([128, 1], F32, tag="mu2")
                        nc.vector.tensor_mul(mu2[:], nmu[:], nmu[:])
                        nc.vector.tensor_sub(var[:], var[:], mu2[:])
                        rstd = stats.tile([128, 1], F32, tag="rstd")
                        nc.scalar.activation(out=rstd[:], in_=var[:], func=Act.Sqrt,
                                             bias=epsl[:], scale=1.0)
                        nc.vector.reciprocal(out=rstd[:], in_=rstd[:])
                        # normed = (pre - mu) * rstd ; * ln_g + ln_b
                        nc.vector.tensor_scalar(
                            out=hm[:], in0=hm[:], scalar1=nmu[:], scalar2=rstd[:],
                            op0=Alu.add, op1=Alu.mult)
                        nc.vector.tensor_mul(hm[:], hm[:], lg[:])
                        nc.vector.tensor_add(hm[:], hm[:], lb[:])
                        # blend: shift = main + is_bag*(emb - main)
                        nc.vector.tensor_sub(hm[:], hm[:], main[:])
                        nc.vector.scalar_tensor_tensor(
                            out=hm[:], in0=hm[:], scalar=isb_m[:, m:m + 1], in1=main[:],
                            op0=Alu.mult, op1=Alu.add)
                        nc.sync.dma_start(out=out_sc[m * 128:(m + 1) * 128, :], in_=hm[:])
                        return hm

                    shifts = [
                        bag_half(u1, w1, ca, ems_a, aco_m, aco_om, a_sc, "a"),
                        bag_half(u2, w2, cv, ems_v, vis_m, vis_om, v_sc, "v")]
                    # h = (o*mask + (1-mask)) * tanh(shift), interleaved so the
                    # tail overlaps the next m-tile's GEMMs
                    for sh, (o_src, m_col, om_col, out_h) in zip(shifts, (
                            (o_scr["a"], aco_m, aco_om, a_h),
                            (o_scr["v"], vis_m, vis_om, v_h))):
                        th = jkp.tile([128, H], F32, tag="jk")
                        nc.scalar.activation(out=th[:], in_=sh[:], func=Act.Tanh)
                        ot = orp.tile([128, H], F32, tag="ot")
                        nc.sync.dma_start(out=ot[:],
                                          in_=o_src[m * 128:(m + 1) * 128, :])
                        nc.vector.tensor_scalar(
                            out=ot[:], in0=ot[:], scalar1=m_col[:, m:m + 1],
                            scalar2=om_col[:, m:m + 1], op0=Alu.mult, op1=Alu.add)
                        nc.vector.tensor_mul(ot[:], ot[:], th[:])
                        nc.sync.dma_start(out=out_h[m * 128:(m + 1) * 128, :], in_=ot[:])

    nc.compile()
    return nc


_NC = None


def _get_nc():
    global _NC
    if _NC is None:
        _NC = build()
    return _NC


BATCH_INPUTS = ("a_x", "a_h0", "a_c0", "v_x", "v_h0", "v_c0",
                "aco_is_rnn_list", "vis_is_rnn_list", "is_bag_list")
F32R_INPUTS = ("a_W", "v_W", "W_mb", "W_b", "b_mb", "b_b")


def _round_f32r(a):
    """Exact float32r rounding (fp32 with 11 explicit mantissa bits, RNE) —
    bitwise-identical to the on-chip DMA/DVE cast (verified on HW)."""
    b = np.ascontiguousarray(a, dtype=np.float32).view(np.uint32)
    lsb = (b >> np.uint32(12)) & np.uint32(1)
    r = (b + np.uint32((1 << 11) - 1) + lsb) & np.uint32(0xFFFFF000)
    return r.view(np.float32)


def kernel(**inputs):
    nc = _get_nc()
    _rounded = {}
    in_maps = []
    for c in range(NCORES):
        im = {}
        for k, v in inputs.items():
            v = np.ascontiguousarray(np.asarray(v), dtype=np.float32)
            if k in F32R_INPUTS:
                v = _rounded.setdefault(k, _round_f32r(v))
            if k in BATCH_INPUTS:
                im[k] = v[c * BL:(c + 1) * BL]
            else:
                im[k] = v
        in_maps.append(im)
    res = run_bass_kernel_spmd(nc, in_maps, list(range(NCORES)))
    outs = res.results
    cat = lambda name: np.concatenate([outs[c][name] for c in range(NCORES)], axis=0)
    return (cat("a_h"), cat("a_sc"), cat("v_h"), cat("v_sc"))

